# revision 3
# baseline (speedup 1.0000x reference)
"""BinaryDense Trainium2 kernel: out = x @ sign(kernel) + bias.

Shapes (hardcoded): x [8192, 4096] f32, kernel [4096, 4096] f32,
bias [4096] f32 -> out [8192, 4096] f32.

Strategy: data-parallel over the 8 NeuronCores -- each core owns a
1024-row slice of x and the full weight matrix.  Per core:
  1. x slice is loaded and transposed on the TensorEngine (identity
     matmul) into a persistent SBUF cache xT [K=4096, 1024] f32.
  2. Weights stream in [128, 512] tiles; sign() runs on the Scalar
     engine; matmuls read both operands as float32r (fp32 bits,
     FP22-reduced multiply - exact for the +/-1 weights, ~1e-4 rel
     error from truncating x, full 1 cycle/row PE throughput).
  3. 8 PSUM banks accumulate the 8 b-tiles of an output column block
     over the 32 k-chunks; DVE adds bias and results DMA out.
"""

import numpy as np
from contextlib import ExitStack

import concourse.bass as bass
import concourse.mybir as mybir
import concourse.tile as tile
from concourse import bacc
from concourse.bass import ts
from concourse.bass_utils import run_bass_kernel_spmd
from concourse.masks import make_identity

B, D_IN, UNITS = 8192, 4096, 4096
N_CORES = 8
ROWS = B // N_CORES  # 1024 rows of x per core

P = 128
N_TILE = 512  # output-column tile (one PSUM bank of f32)

F32 = mybir.dt.float32
F32R = mybir.dt.float32r


def build_body(tc, x, w, bias, out, rows, d_in, units, n_tile=N_TILE):
    nc = tc.nc
    b_tiles = rows // P
    k_tiles = d_in // P
    u_tiles = units // n_tile

    with ExitStack() as ctx:
        const = ctx.enter_context(tc.tile_pool(name="const", bufs=1))
        xt_pool = ctx.enter_context(tc.tile_pool(name="xt", bufs=1))
        stage = ctx.enter_context(tc.tile_pool(name="stage", bufs=2))
        wp = ctx.enter_context(tc.tile_pool(name="wp", bufs=3))
        sp = ctx.enter_context(tc.tile_pool(name="sp", bufs=3))
        op = ctx.enter_context(tc.tile_pool(name="op", bufs=3))

        ident = const.tile([P, P], F32)
        make_identity(nc, ident)

        bias_bc = const.tile([P, units], F32)
        nc.sync.dma_start(bias_bc[:], bias[None, :].to_broadcast([P, units]))

        # Persistent x^T cache: partition = k % 128, dims (k // 128, b).
        # Stored as float32r: the PSUM->SBUF copy rounds to FP22, which the
        # FP32r matmuls require of their producers.
        xt = xt_pool.tile([P, k_tiles, rows], F32R)

        # Phase 1: transpose x into xt via TensorEngine
        with tc.tile_pool(name="tpsum", bufs=2, space="PSUM") as tpsum:
            for bt in range(b_tiles):
                xs = stage.tile([P, d_in], F32, tag="xs")
                nc.sync.dma_start(xs[:], x[ts(bt, P), :])
                for kc in range(k_tiles):
                    pt = tpsum.tile([P, P], F32, tag="pt")
                    nc.tensor.transpose(pt[:], xs[:, ts(kc, P)], ident[:])
                    nc.any.tensor_copy(xt[:, kc, ts(bt, P)], pt[:])

        # Phase 2: main matmul, k-contiguous per output-column block
        with tc.tile_pool(name="mpsum", bufs=b_tiles, space="PSUM") as mpsum:
            for u in range(u_tiles):
                psums = [
                    mpsum.tile([P, n_tile], F32, tag="acc", name=f"acc_{u}_{i}")
                    for i in range(b_tiles)
                ]
                for kc in range(k_tiles):
                    wt = wp.tile([P, n_tile], F32, tag="wt")
                    nc.sync.dma_start(wt[:], w[ts(kc, P), ts(u, n_tile)])
                    st = sp.tile([P, n_tile], F32R, tag="st")
                    nc.scalar.activation(
                        st[:], wt[:], mybir.ActivationFunctionType.Sign
                    )
                    for bt in range(b_tiles):
                        nc.tensor.matmul(
                            psums[bt][:],
                            xt[:, kc, ts(bt, P)],
                            st[:],
                            start=(kc == 0),
                            stop=(kc == k_tiles - 1),
                        )
                for bt in range(b_tiles):
                    ot = op.tile([P, n_tile], F32, tag="ot")
                    nc.vector.tensor_add(
                        ot[:], psums[bt][:], bias_bc[:, ts(u, n_tile)]
                    )
                    nc.sync.dma_start(out[ts(bt, P), ts(u, n_tile)], ot[:])


def build_nc(rows=ROWS, d_in=D_IN, units=UNITS, n_tile=N_TILE):
    nc = bacc.Bacc(
        "TRN2", target_bir_lowering=False, debug=False, num_devices=N_CORES
    )
    x = nc.dram_tensor("x", [rows, d_in], F32, kind="ExternalInput").ap()
    w = nc.dram_tensor("w", [d_in, units], F32, kind="ExternalInput").ap()
    bias = nc.dram_tensor("bias", [units], F32, kind="ExternalInput").ap()
    out = nc.dram_tensor("out", [rows, units], F32, kind="ExternalOutput").ap()
    with tile.TileContext(nc) as tc:
        build_body(tc, x, w, bias, out, rows, d_in, units, n_tile)
    nc.compile()
    return nc


_NC = None


def _get_nc():
    global _NC
    if _NC is None:
        _NC = build_nc()
    return _NC


def run_spmd(x, w, b, trace=False):
    nc = _get_nc()
    in_maps = [
        {"x": np.ascontiguousarray(x[c * ROWS : (c + 1) * ROWS]), "w": w, "bias": b}
        for c in range(N_CORES)
    ]
    res = run_bass_kernel_spmd(
        nc, in_maps, core_ids=list(range(N_CORES)), trace=trace
    )
    out = np.concatenate([res.results[c]["out"] for c in range(N_CORES)], axis=0)
    return out, res


def kernel(x, kernel, bias):
    x = np.ascontiguousarray(x, dtype=np.float32)
    w = np.ascontiguousarray(kernel, dtype=np.float32)
    b = np.ascontiguousarray(bias, dtype=np.float32)
    out, _ = run_spmd(x, w, b)
    return out


# revision 5
# speedup vs baseline: 1.1916x; 1.1916x over previous
"""BinaryDense Trainium2 kernel: out = x @ sign(kernel) + bias.

Shapes (hardcoded): x [8192, 4096] f32, kernel [4096, 4096] f32,
bias [4096] f32 -> out [8192, 4096] f32.

Strategy: data-parallel over the 8 NeuronCores -- each core owns a
1024-row slice of x and the full weight matrix.  The x slice is staged
into device DRAM K-major (transposed during host-side sharding, a pure
layout choice) so the contraction dim lands on SBUF partitions without
any on-device transpose.  Per core:
  1. x^T loads once into a persistent SBUF cache [K=4096, 1024].
  2. Weights stream in [128, 512] tiles; sign() runs on the Scalar
     engine writing float32r (fp32 bits, FP22-reduced multiply --
     exact for the +/-1 weights, ~1e-4 rel error from truncating x,
     full 1 cycle/row PE throughput).
  3. 8 PSUM banks accumulate the 8 row-tiles of an output column
     block over the 32 k-chunks; DVE adds bias; results DMA out.
"""

import numpy as np
from contextlib import ExitStack

import concourse.bass as bass
import concourse.mybir as mybir
import concourse.tile as tile
from concourse import bacc
from concourse.bass import ts
from concourse.bass_utils import run_bass_kernel_spmd

B, D_IN, UNITS = 8192, 4096, 4096
N_CORES = 8
ROWS = B // N_CORES  # 1024 rows of x per core

P = 128
N_TILE = 512  # output-column tile (one PSUM bank of f32)

F32 = mybir.dt.float32
F32R = mybir.dt.float32r


def build_body(tc, xt_dram, w, bias, out, rows, d_in, units, n_tile=N_TILE):
    nc = tc.nc
    b_tiles = rows // P
    k_tiles = d_in // P
    u_tiles = units // n_tile

    with ExitStack() as ctx:
        const = ctx.enter_context(tc.tile_pool(name="const", bufs=1))
        xt_pool = ctx.enter_context(tc.tile_pool(name="xt", bufs=1))
        wp = ctx.enter_context(tc.tile_pool(name="wp", bufs=6))
        sp = ctx.enter_context(tc.tile_pool(name="sp", bufs=6))
        op = ctx.enter_context(tc.tile_pool(name="op", bufs=3))

        bias_bc = const.tile([P, units], F32)
        nc.sync.dma_start(bias_bc[:], bias[None, :].to_broadcast([P, units]))

        # Persistent x^T cache: partition = k % 128, dims (k // 128, b).
        # float32r so the FP32r matmuls accept it as a pre-rounded operand.
        xt = xt_pool.tile([P, k_tiles, rows], F32R)
        # xt_dram rows are k; row ko*128+ki -> partition ki, free (ko, b)
        xt_src = xt_dram.bitcast(F32R).rearrange("(ko ki) b -> ki ko b", ki=P)
        for kq in range(4):
            q = k_tiles // 4
            nc.sync.dma_start(xt[:, ts(kq, q), :], xt_src[:, ts(kq, q), :])

        with tc.tile_pool(name="mpsum", bufs=b_tiles, space="PSUM") as mpsum:
            for u in range(u_tiles):
                psums = [
                    mpsum.tile([P, n_tile], F32, tag="acc", name=f"acc_{u}_{i}")
                    for i in range(b_tiles)
                ]
                for kc in range(k_tiles):
                    wt = wp.tile([P, n_tile], F32, tag="wt")
                    nc.sync.dma_start(wt[:], w[ts(kc, P), ts(u, n_tile)])
                    st = sp.tile([P, n_tile], F32R, tag="st")
                    nc.scalar.activation(
                        st[:], wt[:], mybir.ActivationFunctionType.Sign
                    )
                    for bt in range(b_tiles):
                        nc.tensor.matmul(
                            psums[bt][:],
                            xt[:, kc, ts(bt, P)],
                            st[:],
                            start=(kc == 0),
                            stop=(kc == k_tiles - 1),
                        )
                for bt in range(b_tiles):
                    ot = op.tile([P, n_tile], F32, tag="ot")
                    nc.vector.tensor_add(
                        ot[:], psums[bt][:], bias_bc[:, ts(u, n_tile)]
                    )
                    nc.sync.dma_start(out[ts(bt, P), ts(u, n_tile)], ot[:])


def build_nc(rows=ROWS, d_in=D_IN, units=UNITS, n_tile=N_TILE):
    nc = bacc.Bacc(
        "TRN2", target_bir_lowering=False, debug=False, num_devices=N_CORES
    )
    xt = nc.dram_tensor("xt", [d_in, rows], F32R, kind="ExternalInput").ap()
    w = nc.dram_tensor("w", [d_in, units], F32, kind="ExternalInput").ap()
    bias = nc.dram_tensor("bias", [units], F32, kind="ExternalInput").ap()
    out = nc.dram_tensor("out", [rows, units], F32, kind="ExternalOutput").ap()
    with tile.TileContext(nc) as tc:
        build_body(tc, xt, w, bias, out, rows, d_in, units, n_tile)
    nc.compile()
    return nc


_NC = None


def _get_nc():
    global _NC
    if _NC is None:
        _NC = build_nc()
    return _NC


def run_spmd(x, w, b, trace=False):
    nc = _get_nc()
    in_maps = [
        {
            "xt": np.ascontiguousarray(x[c * ROWS : (c + 1) * ROWS].T),
            "w": w,
            "bias": b,
        }
        for c in range(N_CORES)
    ]
    res = run_bass_kernel_spmd(
        nc, in_maps, core_ids=list(range(N_CORES)), trace=trace
    )
    out = np.concatenate([res.results[c]["out"] for c in range(N_CORES)], axis=0)
    return out, res


def kernel(x, kernel, bias):
    x = np.ascontiguousarray(x, dtype=np.float32)
    w = np.ascontiguousarray(kernel, dtype=np.float32)
    b = np.ascontiguousarray(bias, dtype=np.float32)
    out, _ = run_spmd(x, w, b)
    return out


# revision 6
# speedup vs baseline: 1.2361x; 1.0374x over previous
"""BinaryDense Trainium2 kernel: out = x @ sign(kernel) + bias.

Shapes (hardcoded): x [8192, 4096] f32, kernel [4096, 4096] f32,
bias [4096] f32 -> out [8192, 4096] f32.

Strategy: data-parallel over the 8 NeuronCores -- each core owns a
1024-row slice of x and the full weight matrix.  The x slice is staged
into device DRAM K-major (transposed during host-side sharding, a pure
layout choice) so the contraction dim lands on SBUF partitions without
any on-device transpose.  Per core:
  1. x^T loads once into a persistent SBUF cache [K=4096, 1024].
  2. Weights stream in [128, 512] tiles; sign() runs on the Scalar
     engine writing float32r (fp32 bits, FP22-reduced multiply --
     exact for the +/-1 weights, ~1e-4 rel error from truncating x,
     full 1 cycle/row PE throughput).
  3. 8 PSUM banks accumulate the 8 row-tiles of an output column
     block over the 32 k-chunks; DVE adds bias; results DMA out.
"""

import numpy as np
from contextlib import ExitStack

import concourse.bass as bass
import concourse.mybir as mybir
import concourse.tile as tile
from concourse import bacc
from concourse.bass import ts
from concourse.bass_utils import run_bass_kernel_spmd

B, D_IN, UNITS = 8192, 4096, 4096
N_CORES = 8
ROWS = B // N_CORES  # 1024 rows of x per core

P = 128
N_TILE = 512  # output-column tile (one PSUM bank of f32)

F32 = mybir.dt.float32
F32R = mybir.dt.float32r


def build_body(tc, xt_dram, w, bias, out, rows, d_in, units, n_tile=N_TILE):
    nc = tc.nc
    b_tiles = rows // P
    k_tiles = d_in // P
    u_tiles = units // n_tile

    with ExitStack() as ctx:
        const = ctx.enter_context(tc.tile_pool(name="const", bufs=1))
        xt_pool = ctx.enter_context(tc.tile_pool(name="xt", bufs=1))
        wp = ctx.enter_context(tc.tile_pool(name="wp", bufs=6))
        sp = ctx.enter_context(tc.tile_pool(name="sp", bufs=6))
        op = ctx.enter_context(tc.tile_pool(name="op", bufs=3))

        # Persistent x^T cache: partition = k % 128, dims (k // 128, b).
        # float32r so the FP32r matmuls accept it as a pre-rounded operand.
        # One DMA per k-chunk (512 KB), on SWDGE so the first chunks land
        # fast without queuing behind the HWDGE weight stream.
        xt = xt_pool.tile([P, k_tiles, rows], F32R)
        # xt_dram rows are k; row ko*128+ki -> partition ki, free (ko, b)
        xt_src = xt_dram.bitcast(F32R).rearrange("(ko ki) b -> ki ko b", ki=P)
        for ko in range(k_tiles):
            nc.gpsimd.dma_start(xt[:, ko, :], xt_src[:, ko, :])

        bias_bc = const.tile([P, units], F32)
        nc.gpsimd.dma_start(bias_bc[:], bias[None, :].to_broadcast([P, units]))

        with tc.tile_pool(name="mpsum", bufs=b_tiles, space="PSUM") as mpsum:
            for u in range(u_tiles):
                psums = [
                    mpsum.tile([P, n_tile], F32, tag="acc", name=f"acc_{u}_{i}")
                    for i in range(b_tiles)
                ]
                for kc in range(k_tiles):
                    wt = wp.tile([P, n_tile], F32, tag="wt")
                    nc.sync.dma_start(wt[:], w[ts(kc, P), ts(u, n_tile)])
                    st = sp.tile([P, n_tile], F32R, tag="st")
                    nc.scalar.activation(
                        st[:], wt[:], mybir.ActivationFunctionType.Sign
                    )
                    for bt in range(b_tiles):
                        nc.tensor.matmul(
                            psums[bt][:],
                            xt[:, kc, ts(bt, P)],
                            st[:],
                            start=(kc == 0),
                            stop=(kc == k_tiles - 1),
                        )
                for bt in range(b_tiles):
                    ot = op.tile([P, n_tile], F32, tag="ot")
                    nc.vector.tensor_add(
                        ot[:], psums[bt][:], bias_bc[:, ts(u, n_tile)]
                    )
                    nc.sync.dma_start(out[ts(bt, P), ts(u, n_tile)], ot[:])


def build_nc(rows=ROWS, d_in=D_IN, units=UNITS, n_tile=N_TILE):
    nc = bacc.Bacc(
        "TRN2", target_bir_lowering=False, debug=False, num_devices=N_CORES
    )
    xt = nc.dram_tensor("xt", [d_in, rows], F32R, kind="ExternalInput").ap()
    w = nc.dram_tensor("w", [d_in, units], F32, kind="ExternalInput").ap()
    bias = nc.dram_tensor("bias", [units], F32, kind="ExternalInput").ap()
    out = nc.dram_tensor("out", [rows, units], F32, kind="ExternalOutput").ap()
    with tile.TileContext(nc) as tc:
        build_body(tc, xt, w, bias, out, rows, d_in, units, n_tile)
    nc.compile()
    return nc


_NC = None


def _get_nc():
    global _NC
    if _NC is None:
        _NC = build_nc()
    return _NC


def run_spmd(x, w, b, trace=False):
    nc = _get_nc()
    in_maps = [
        {
            "xt": np.ascontiguousarray(x[c * ROWS : (c + 1) * ROWS].T),
            "w": w,
            "bias": b,
        }
        for c in range(N_CORES)
    ]
    res = run_bass_kernel_spmd(
        nc, in_maps, core_ids=list(range(N_CORES)), trace=trace
    )
    out = np.concatenate([res.results[c]["out"] for c in range(N_CORES)], axis=0)
    return out, res


def kernel(x, kernel, bias):
    x = np.ascontiguousarray(x, dtype=np.float32)
    w = np.ascontiguousarray(kernel, dtype=np.float32)
    b = np.ascontiguousarray(bias, dtype=np.float32)
    out, _ = run_spmd(x, w, b)
    return out


# revision 7
# speedup vs baseline: 1.2634x; 1.0221x over previous
"""BinaryDense Trainium2 kernel: out = x @ sign(kernel) + bias.

Shapes (hardcoded): x [8192, 4096] f32, kernel [4096, 4096] f32,
bias [4096] f32 -> out [8192, 4096] f32.

Strategy: data-parallel over the 8 NeuronCores -- each core owns a
1024-row slice of x and the full weight matrix.  The x slice is staged
into device DRAM K-major (transposed during host-side sharding, a pure
layout choice) so the contraction dim lands on SBUF partitions without
any on-device transpose.  Per core:
  1. x^T loads once into a persistent SBUF cache [K=4096, 1024].
  2. Weights stream in [128, 512] tiles; sign() runs on the Scalar
     engine writing float32r (fp32 bits, FP22-reduced multiply --
     exact for the +/-1 weights, ~1e-4 rel error from truncating x,
     full 1 cycle/row PE throughput).
  3. 8 PSUM banks accumulate the 8 row-tiles of an output column
     block over the 32 k-chunks; DVE adds bias; results DMA out.
"""

import numpy as np
from contextlib import ExitStack

import concourse.bass as bass
import concourse.mybir as mybir
import concourse.tile as tile
from concourse import bacc
from concourse.bass import ts
from concourse.bass_utils import run_bass_kernel_spmd

B, D_IN, UNITS = 8192, 4096, 4096
N_CORES = 8
ROWS = B // N_CORES  # 1024 rows of x per core

P = 128
N_TILE = 512  # output-column tile (one PSUM bank of f32)

F32 = mybir.dt.float32
F32R = mybir.dt.float32r


def build_body(tc, xt_dram, w, bias, out, rows, d_in, units, n_tile=N_TILE):
    nc = tc.nc
    b_tiles = rows // P
    k_tiles = d_in // P
    u_tiles = units // n_tile

    with ExitStack() as ctx:
        const = ctx.enter_context(tc.tile_pool(name="const", bufs=1))
        xt_pool = ctx.enter_context(tc.tile_pool(name="xt", bufs=1))
        wp = ctx.enter_context(tc.tile_pool(name="wp", bufs=6))
        sp = ctx.enter_context(tc.tile_pool(name="sp", bufs=6))
        op = ctx.enter_context(tc.tile_pool(name="op", bufs=3))

        # Persistent x^T cache: partition = k % 128, dims (k // 128, b).
        # float32r so the FP32r matmuls accept it as a pre-rounded operand.
        # One DMA per k-chunk (512 KB); chunk DMAs are interleaved with the
        # first u-iteration's weight-tile DMAs on the same HWDGE queue so
        # both streams arrive just-in-time (the first pass is DMA-
        # oversubscribed: first-touch demand ~420 GB/s vs ~350 available).
        xt = xt_pool.tile([P, k_tiles, rows], F32R)
        # xt_dram rows are k; row ko*128+ki -> partition ki, free (ko, b)
        xt_src = xt_dram.bitcast(F32R).rearrange("(ko ki) b -> ki ko b", ki=P)

        def load_xt(ko):
            nc.sync.dma_start(xt[:, ko, :], xt_src[:, ko, :])

        bias_bc = const.tile([P, units], F32)

        with tc.tile_pool(name="mpsum", bufs=b_tiles, space="PSUM") as mpsum:
            for u in range(u_tiles):
                psums = [
                    mpsum.tile([P, n_tile], F32, tag="acc", name=f"acc_{u}_{i}")
                    for i in range(b_tiles)
                ]
                for kc in range(k_tiles):
                    if u == 0:
                        # keep the x^T stream exactly 2 chunks ahead
                        for ko in [0, 1, 2] if kc == 0 else [kc + 2]:
                            if ko < k_tiles:
                                load_xt(ko)
                    wt = wp.tile([P, n_tile], F32, tag="wt")
                    nc.sync.dma_start(wt[:], w[ts(kc, P), ts(u, n_tile)])
                    st = sp.tile([P, n_tile], F32R, tag="st")
                    nc.scalar.activation(
                        st[:], wt[:], mybir.ActivationFunctionType.Sign
                    )
                    for bt in range(b_tiles):
                        nc.tensor.matmul(
                            psums[bt][:],
                            xt[:, kc, ts(bt, P)],
                            st[:],
                            start=(kc == 0),
                            stop=(kc == k_tiles - 1),
                        )
                if u == 0:
                    # bias is first needed at u=0's drain; load it after the
                    # oversubscribed first k-sweep
                    nc.sync.dma_start(
                        bias_bc[:], bias[None, :].to_broadcast([P, units])
                    )
                for bt in range(b_tiles):
                    ot = op.tile([P, n_tile], F32, tag="ot")
                    nc.vector.tensor_add(
                        ot[:], psums[bt][:], bias_bc[:, ts(u, n_tile)]
                    )
                    # out goes on the Activation HWDGE queue, off the
                    # input-stream (SP) queue
                    nc.scalar.dma_start(out[ts(bt, P), ts(u, n_tile)], ot[:])


def build_nc(rows=ROWS, d_in=D_IN, units=UNITS, n_tile=N_TILE):
    nc = bacc.Bacc(
        "TRN2", target_bir_lowering=False, debug=False, num_devices=N_CORES
    )
    xt = nc.dram_tensor("xt", [d_in, rows], F32R, kind="ExternalInput").ap()
    w = nc.dram_tensor("w", [d_in, units], F32, kind="ExternalInput").ap()
    bias = nc.dram_tensor("bias", [units], F32, kind="ExternalInput").ap()
    out = nc.dram_tensor("out", [rows, units], F32, kind="ExternalOutput").ap()
    with tile.TileContext(nc) as tc:
        build_body(tc, xt, w, bias, out, rows, d_in, units, n_tile)
    nc.compile()
    return nc


_NC = None


def _get_nc():
    global _NC
    if _NC is None:
        _NC = build_nc()
    return _NC


def run_spmd(x, w, b, trace=False):
    nc = _get_nc()
    in_maps = [
        {
            "xt": np.ascontiguousarray(x[c * ROWS : (c + 1) * ROWS].T),
            "w": w,
            "bias": b,
        }
        for c in range(N_CORES)
    ]
    res = run_bass_kernel_spmd(
        nc, in_maps, core_ids=list(range(N_CORES)), trace=trace
    )
    out = np.concatenate([res.results[c]["out"] for c in range(N_CORES)], axis=0)
    return out, res


def kernel(x, kernel, bias):
    x = np.ascontiguousarray(x, dtype=np.float32)
    w = np.ascontiguousarray(kernel, dtype=np.float32)
    b = np.ascontiguousarray(bias, dtype=np.float32)
    out, _ = run_spmd(x, w, b)
    return out
